# revision 46
# baseline (speedup 1.0000x reference)
"""Trainium2 Bass kernel for the Cocoa contrastive loss.

loss = mean_i exp((1 - cos(x_i, y_i))/tau)
     + sum_{i in neg, j not in neg} exp(cos(x_i, x_j)/tau) / cnt
     + sum_{i in neg, j not in neg} exp(cos(y_i, y_j)/tau) / cnt

with neg = rows whose label has > 32 zeros, cnt = n_neg * n_nonneg.

Strategy (8 NeuronCores):
  Host: neg mask (exact integer math), l2-normalize in f32, pos term in
        float64, fp8(e4m3) quantization and the transposed [D, rows]
        layouts the GEMM wants.  All O(B*D) work; the O(B^2*D) part runs
        on the device.
  Device (single SPMD launch, 4x2 grid over neg x nonneg rows): per core
        a [m_loc x n_loc] slab of each Gram sim = Z_neg @ Z_nonneg^T with
        K=D on partitions, fp8 DoubleRow matmuls (the 157 TF/s sustained
        peak; the matmul stream fully hides LDWEIGHTS), exp(sim/tau) on
        ScalarE with per-partition accumulation into a [128, 2*n_ch]
        column vector per core.
  Host: combine partial sums (subtract the exp(0)=1 contributions of the
        zero padding), add the pos term.

Launch timeline (per core, measured): ~7.2us fixed framework preamble,
then DMA triggers on both HWDGE rings (sync + scalar) bring in tile 0's
~2.45MB prefix by ~12.9us while warmup matmuls on a memset tile hold the
PE's p-state governor at speed (it ramps only under sustained matmul
streaming and decays across multi-us idle gaps); the 288 real matmuls
then stream gaplessly at 196ns each (~56.5us, >99% dense); ~5us tail
(last exp + acc DMA + fixed teardown barriers).  Total ~75us vs the
two-phase baseline's ~186us.
"""

import numpy as np
import ml_dtypes

import concourse.bass as bass
import concourse.bacc as bacc
import concourse.mybir as mybir
import concourse.tile as tile
from concourse.bass_utils import run_bass_kernel_spmd

TAU = 0.1
THRESHOLD = 32
B, D, L = 4096, 4096, 64
NCORES = 8
KCH = D // 128      # 32 contraction chunks of 128
KSP = KCH // 2      # 16 matmul steps (DoubleRow: 2 chunks per matmul)
A_SPLIT, B_SPLIT = 4, 2  # core grid over (neg rows, nonneg rows)

F32 = mybir.dt.float32
BF16 = mybir.dt.bfloat16
FP8 = mybir.dt.float8e4
FP8_NP = ml_dtypes.float8_e4m3fn
FP8_SCALE = 24.0  # centers N(0, 1/4096) values in e4m3's normal range

# module-level caches so repeated kernel() calls don't rebuild/recompile
_CACHE: dict = {}

# filled in by the last kernel() call when tracing is enabled (test harness use)
LAST_RESULTS: list = []


def _build_gram(m_loc: int, n_ch: int) -> bass.Bass:
    """Per-core fp8 DoubleRow GEMM: [m_loc neg rows] x [n_ch*128 nonneg rows].

    The nonneg side is the 128-wide stationary operand, the neg side the
    m_loc-wide moving operand, so the matmul stream hides LDWEIGHTS.
    Each dma_start costs ~650ns of serial Sync-sequencer descriptor time,
    so loads are batched into few triggers: 1 per stationary tile, 4 for
    the first moving tile (so the PE can start on the first k-quarter),
    1 for the second moving tile.
    Host-supplied layouts:
      l{x,y}: [128, KSP, 2, m_loc]   moving side (neg rows)
      r{x,y}: [n_ch, 128, KCH, 128]  stationary side (nonneg rows)
    """
    nc = bacc.Bacc(None)
    lx = nc.declare_dram_parameter("lx", [128, KSP, 2, m_loc], FP8, isOutput=False)
    ly = nc.declare_dram_parameter("ly", [128, KSP, 2, m_loc], FP8, isOutput=False)
    rx = nc.declare_dram_parameter("rx", [n_ch, 128, KCH, 128], FP8, isOutput=False)
    ry = nc.declare_dram_parameter("ry", [n_ch, 128, KCH, 128], FP8, isOutput=False)
    # last column is warmup junk (host ignores it)
    acc_out = nc.declare_dram_parameter("acc", [128, 2 * n_ch + 1], F32,
                                        isOutput=True)

    with tile.TileContext(nc) as tc:
        with (
            tc.tile_pool(name="mov", bufs=1) as movp,
            tc.tile_pool(name="sta", bufs=1) as stap,
            tc.tile_pool(name="ps", bufs=4, space="PSUM") as psp,
            tc.tile_pool(name="junk", bufs=4) as junkp,
            tc.tile_pool(name="accp", bufs=1) as accp,
        ):
            acc = accp.tile([128, 2 * n_ch + 1], F32)
            lt = {
                "x": movp.tile([128, KSP, 2, m_loc], FP8, tag="ltx", name="lt_x"),
                "y": movp.tile([128, KSP, 2, m_loc], FP8, tag="lty", name="lt_y"),
            }
            # all stationary tiles live for the whole kernel (distinct tags:
            # no buffer rotation): SBUF is big enough and this avoids
            # pool-reuse false dependencies on the loads
            st = {
                (nm, c): stap.tile([128, KCH, 128], FP8, tag=f"st{nm}{c}",
                                   name=f"st_{nm}{c}")
                for nm in ("x", "y")
                for c in range(n_ch)
            }

            # DMA issue in first-use order across the two HWDGE trigger
            # rings (each dma_start costs ~650ns of serial sequencer time).
            # Tile 0's prefix (st_x0 + lt_x, ~2.45MB) is HBM-bandwidth-bound
            # and lands ~12-13us in; lt_x is split in 4 k-quarters so its
            # matmuls can begin on the first quarter.  Ring-cumulative bytes
            # complete in kp-need order: scalar carries q0 then q1
            # (0.47/0.94MB), sync st_x0 then q2, q3 (0.59/1.06/1.53MB).
            # lt_y (needed only at ~40us) is parked behind the first few
            # stationary tiles.
            def ltq(nm, g):
                return dict(out=lt[nm][:, 4 * g:4 * g + 4],
                            in_=(lx if nm == "x" else ly)[:, 4 * g:4 * g + 4])
            # st_x0 split in K-halves so the rings carry equal tile0 bytes
            # (1.24MB each) with per-ring completions in kp-need order:
            # sync [front(kp0), q2(kp8), q3(kp12)], scalar [q0(kp0),
            # q1(kp4), back(kp8)]
            nc.sync.dma_start(out=st["x", 0][:, :16, :], in_=rx[0, :, :16, :])
            nc.scalar.dma_start(**ltq("x", 0))
            nc.scalar.dma_start(**ltq("x", 1))
            nc.sync.dma_start(**ltq("x", 2))
            nc.sync.dma_start(**ltq("x", 3))
            nc.scalar.dma_start(out=st["x", 0][:, 16:, :], in_=rx[0, :, 16:, :])
            for c in range(1, n_ch):
                nc.sync.dma_start(out=st["x", c], in_=rx[c])
                if c == min(3, n_ch - 1):
                    nc.sync.dma_start(out=lt["y"], in_=ly[:])
            if n_ch == 1:
                nc.sync.dma_start(out=lt["y"], in_=ly[:])
            for c in range(n_ch):
                nc.sync.dma_start(out=st["y", c], in_=ry[c])

            # PE p-state (HAM) warmup: the governor only ramps under
            # sustained matmul STREAMING (LDWEIGHTS-bound dummies don't
            # count) and de-ramps across multi-us idle gaps, so run a 224-row
            # dummy stream (just enough to hide the 135ns LDWEIGHTS) from a
            # single small memset tile used as both operands, sized to end
            # right when tile 0's prefix typically lands.  Without this the
            # first ~45 real matmuls run at 387-418ns instead of 196ns.
            wmov = movp.tile([128, 2, 224], FP8, tag="wmov", name="wmov")
            nc.vector.memset(wmov, 0.0)
            wps = psp.tile([128, 224], F32, tag="wps", bufs=1)
            for _ in range(26):
                nc.tensor.matmul(
                    wps, lhsT=wmov[:, :, :128], rhs=wmov, start=True,
                    stop=True, perf_mode=mybir.MatmulPerfMode.DoubleRow)
            wj = junkp.tile([128, 224], BF16, tag="junk")
            nc.scalar.activation(
                wj, wps, mybir.ActivationFunctionType.Exp,
                scale=1.0 / (TAU * FP8_SCALE * FP8_SCALE),
                accum_out=acc[:, 2 * n_ch:2 * n_ch + 1])

            col = 0
            for nm in ("x", "y"):
                for c in range(n_ch):
                    s_t = st[nm, c]
                    ps = psp.tile([128, m_loc], F32, tag="ps")
                    for kp in range(KSP):
                        nc.tensor.matmul(
                            ps,
                            lhsT=s_t[:, 2 * kp:2 * kp + 2, :],
                            rhs=lt[nm][:, kp, :, :],
                            start=(kp == 0), stop=(kp == KSP - 1),
                            perf_mode=mybir.MatmulPerfMode.DoubleRow)
                    j = junkp.tile([128, m_loc], BF16, tag="junk")
                    nc.scalar.activation(
                        j, ps,
                        mybir.ActivationFunctionType.Exp,
                        scale=1.0 / (TAU * FP8_SCALE * FP8_SCALE),
                        accum_out=acc[:, col:col + 1])
                    col += 1
            # result DMA triggered from the scalar ring: the last exp ran on
            # the scalar engine, so no cross-engine semaphore hop
            nc.scalar.dma_start(out=acc_out[:], in_=acc)
    nc.compile()
    return nc


def _run_spmd(key, builder, in_maps):
    import os
    if key not in _CACHE:
        _CACHE[key] = builder()
    nc = _CACHE[key]
    trace = bool(os.environ.get("COCOA_TRACE"))
    res = run_bass_kernel_spmd(nc, in_maps, list(range(NCORES)), trace=trace)
    LAST_RESULTS.append((key, res))
    return res.results


def kernel(x_pred_batch: np.ndarray, y_pred_batch: np.ndarray,
           label_batch: np.ndarray) -> np.ndarray:
    x = np.ascontiguousarray(x_pred_batch, dtype=np.float32)
    y = np.ascontiguousarray(y_pred_batch, dtype=np.float32)
    lab = np.asarray(label_batch)

    # exact mask on host
    zero_counts = (lab == 0).sum(axis=1)
    neg_mask = zero_counts > THRESHOLD
    n1 = int(neg_mask.sum())
    n2 = B - n1
    cnt = n1 * n2

    # l2-normalize; pos term in float64
    xn = x / np.linalg.norm(x, axis=1, keepdims=True)
    yn = y / np.linalg.norm(y, axis=1, keepdims=True)
    cos_pos = np.einsum("ij,ij->i", xn.astype(np.float64), yn.astype(np.float64))
    pos_error = float(np.mean(np.exp((1.0 - cos_pos) / TAU)))

    if cnt == 0:
        return np.float32(pos_error)

    # DoubleRow's moving AP requires the inner step (m_loc elements) to be
    # a multiple of 16 (ISA: step%16==0), and one PSUM bank caps it at 512
    m_loc = 16 * max(1, -(-n1 // (A_SPLIT * 16)))
    n_loc = 128 * max(1, -(-n2 // (B_SPLIT * 128)))
    assert m_loc <= 512, f"{m_loc=} exceeds one PSUM bank per tile"
    n1p, n2p = A_SPLIT * m_loc, B_SPLIT * n_loc
    n_ch = n_loc // 128
    n_ch_tot = B_SPLIT * n_ch

    padded = {}
    for nm, zn in (("x", xn), ("y", yn)):
        q = (zn * FP8_SCALE).astype(FP8_NP)  # [B, D]
        lhs = np.zeros((D, n1p), FP8_NP)
        lhs[:, :n1] = q[neg_mask].T
        rhs = np.zeros((D, n2p), FP8_NP)
        rhs[:, :n2] = q[~neg_mask].T
        # moving: [128, KSP, 2, n1p]; element (p, k, r, m) = Z[(2k+r)*128+p, m]
        padded["l" + nm] = np.ascontiguousarray(
            lhs.reshape(KSP, 2, 128, n1p).transpose(2, 0, 1, 3))
        # stationary: [n_ch_tot, 128, KCH, 128];
        # element (c, p, j, i) = Z[j*128+p, 128c+i]
        padded["r" + nm] = np.ascontiguousarray(
            rhs.reshape(KCH, 128, n_ch_tot, 128).transpose(2, 1, 0, 3))

    in_maps = []
    for c in range(NCORES):
        a, bgrid = divmod(c, B_SPLIT)
        cmap = {}
        for nm in ("x", "y"):
            cmap["l" + nm] = np.ascontiguousarray(
                padded["l" + nm][..., a * m_loc:(a + 1) * m_loc])
            cmap["r" + nm] = padded["r" + nm][bgrid * n_ch:(bgrid + 1) * n_ch]
        in_maps.append(cmap)

    res = _run_spmd(("gram", m_loc, n_ch), lambda: _build_gram(m_loc, n_ch),
                    in_maps)

    sx = sy = 0.0
    for r in res:
        acc = r["acc"].astype(np.float64)
        sx += acc[:, :n_ch].sum()
        sy += acc[:, n_ch:2 * n_ch].sum()
    pad = float(n1p) * n2p - float(n1) * n2
    neg_total = ((sx - pad) + (sy - pad)) / cnt

    return np.float32(pos_error + neg_total)
